# revision 1
# baseline (speedup 1.0000x reference)
"""Trainium2 Bass kernel for nn_DescriptionAware (dense_mlp).

Self-contained: takes FULL inputs (as in reference.setup_inputs()), shards
across 8 NeuronCores (batch x class-half), runs one SPMD Bass/Tile program,
reassembles the full [B,S,C] f32 logits on host.

Sharding: core k handles batch b=k//2 and classes [32*(k%2), 32*(k%2)+32).

v2: all embedding gathers go through dma_gather (994ns fixed cost per SWDGE
instruction vs 34 indirect DMAs before), word_emb split lo/hi to fit int16
indices, bf16 operands throughout, host-precomputed masks/lengths/planes.
"""

import os
import numpy as np
import ml_dtypes

import concourse.bass as bass
import concourse.mybir as mybir
import concourse.tile as tile
from concourse import bacc
from concourse.bass_utils import run_bass_kernel_spmd
from concourse.tile_rust import add_dep_helper

# problem dims (hardcoded per contract)
B, S, H = 4, 256, 768
C = 64
LD = 128
E = 300
NS = 8
LP = 32
LA = 16
V = 50000
DH = 300

NCORES = 8
CH = 32                      # classes per core
VSPLIT = 32768               # word_emb row split for int16 gather indices
ES = 384                     # padded embedding row (768B, %256=0)
DCH = [(0, 128), (128, 256), (256, 300)]   # d-chunks of DH=300
HCH = 6                      # 768 = 6*128
KLR = [128, 128, 128, 44]    # w1l row chunks (LD then E in 128s)
KA = [128] * 8 + [45]        # wa1_aug row chunks (1068+1 bias row)

F32 = mybir.dt.float32
BF16 = mybir.dt.bfloat16
I16 = mybir.dt.int16
AL = mybir.AluOpType
AF = mybir.ActivationFunctionType

BF = ml_dtypes.bfloat16

# cf32 const/param column layout ([128, CF_N] f32)
CF_ONES = 0        # 8 cols, all ones (rows used: [0:1] and [0:8])
CF_ID8 = 8         # 8 cols, rows 0:8 identity
CF_SCOL = 16       # 1 col, rows 0:8: 0 or -1e5 per sense
CF_BA2 = 17        # 1 col, rows 0:8: ba2
CF_B2B = 18        # 1 col, all rows: b2
CF_B1R = 19        # 300 cols, row 0: b1
CF_O8x128 = 320    # 128 cols, rows 0:8 all ones (W8b broadcast lhsT)
CF_N = 448


def _pack(a, rows, cols):
    # [k*128, cols] -> [128, k*cols] p-major
    k = rows // 128
    return np.ascontiguousarray(
        a.reshape(k, 128, cols).transpose(1, 0, 2).reshape(128, k * cols))


def _wrap_idx(flat):
    """[n] int -> [128, n//16] int16, slot i at (i%16, i//16), replicated."""
    n = len(flat)
    a = np.zeros((128, n // 16), np.int16)
    a[np.arange(n) % 16, np.arange(n) // 16] = flat
    for r in range(1, 8):
        a[16 * r:16 * (r + 1), :] = a[0:16, :]
    return a


def prepare(inputs):
    """Host-side packing. Returns (dims, in_maps)."""
    x = np.asarray(inputs["x"], np.float32)
    pred_start = np.asarray(inputs["pred_start"]).astype(np.int64)
    pred_end = np.asarray(inputs["pred_end"]).astype(np.int64)
    pdi = np.asarray(inputs["pred_desc_ids"]).astype(np.int64)
    adi = np.asarray(inputs["arg_desc_ids"]).astype(np.int64)
    label_emb = np.asarray(inputs["label_emb"], np.float32)
    word_emb = np.asarray(inputs["word_emb"], np.float32)
    Wa1 = np.asarray(inputs["Wa1"], np.float32)
    ba1 = np.asarray(inputs["ba1"], np.float32)
    Wa2 = np.asarray(inputs["Wa2"], np.float32)
    ba2 = np.asarray(inputs["ba2"], np.float32)
    W1 = np.ascontiguousarray(np.asarray(inputs["W1"], np.float32))
    b1 = np.asarray(inputs["b1"], np.float32)
    W2 = np.asarray(inputs["W2"], np.float32).reshape(DH)
    b2 = np.asarray(inputs["b2"], np.float32)

    # ---- shared packs ----
    wtab = np.zeros((V, ES), BF)
    wtab[:, :E] = word_emb.astype(BF)
    wlo = np.ascontiguousarray(wtab[:VSPLIT])
    whi = np.ascontiguousarray(wtab[VSPLIT:])

    wa1_aug = np.zeros((1152, H), np.float32)
    wa1_aug[:1068] = Wa1
    wa1_aug[1068] = ba1
    wa1_p = _pack(wa1_aug, 1152, H).astype(BF)

    w1x_p = _pack(W1[0:768], 768, DH).astype(BF)
    w1l_f = np.zeros((512, DH), np.float32)
    w1l_f[:428] = W1[768:1196]
    w1l_p = _pack(w1l_f, 512, DH).astype(BF)
    w1p_p = _pack(np.ascontiguousarray(W1[1196:1964]), 768, DH).astype(BF)

    w2f = np.zeros((384, 32), np.float32)
    w2f[:DH, 0] = W2
    w2_p = _pack(w2f, 384, 32).astype(BF)

    # ---- per-core slot streams ----
    # pd: (idx, sense, weight); arg[cb]: (idx, c8, sense, weight)
    core_pd = []   # (lo_list, hi_list)
    core_arg = []  # [cb][lo/hi] lists
    core_scol = []
    for core in range(NCORES):
        b, ch = core // 2, core % 2
        plen = (pdi[b] > 0).sum(-1)          # [8]
        pl, ph = [], []
        for n in range(NS):
            w_ = 1.0 / max(1, int(plen[n]))
            for l in range(LP):
                idv = int(pdi[b, n, l])
                if idv > 0:
                    if idv < VSPLIT:
                        pl.append((idv, n, w_))
                    else:
                        ph.append((idv - VSPLIT, n, w_))
        core_pd.append((pl, ph))
        core_scol.append(np.where(plen > 0, 0.0, -1e5).astype(np.float32))

        ids = adi[b, :, ch * CH:(ch + 1) * CH, :]     # [8, 32, 16]
        alen = np.maximum(1, (ids > 0).sum(-1))       # [8, 32]
        ab = [[[], []] for _ in range(4)]
        for n in range(NS):
            for c in range(CH):
                w_ = 1.0 / float(alen[n, c])
                cb, c8 = c // 8, c % 8
                for l in range(LA):
                    idv = int(ids[n, c, l])
                    if idv > 0:
                        if idv < VSPLIT:
                            ab[cb][0].append((idv, c8, n, w_))
                        else:
                            ab[cb][1].append((idv - VSPLIT, c8, n, w_))
        core_arg.append(ab)

    cdiv = lambda a, b: -(-a // b)
    vPlo = max(1, max(len(core_pd[c][0]) for c in range(NCORES)))
    vPhi = max(1, max(len(core_pd[c][1]) for c in range(NCORES)))
    nPlo, nPhi = cdiv(vPlo, 128), cdiv(vPhi, 128)
    vAlo = [max(1, max(len(core_arg[c][cb][0]) for c in range(NCORES)))
            for cb in range(4)]
    vAhi = [max(1, max(len(core_arg[c][cb][1]) for c in range(NCORES)))
            for cb in range(4)]
    nAlo = [cdiv(v, 128) for v in vAlo]
    nAhi = [cdiv(v, 128) for v in vAhi]
    # fewer distinct num_idxs_reg values -> fewer ~400ns Pool-sequencer MOVEs
    # in the gather-dispatch prelude (chunk counts unchanged: capped per section)
    vAlo = [min(nAlo[cb] * 128, max(vAlo)) for cb in range(4)]
    vAhi = [min(nAhi[cb] * 128, max(vAhi)) for cb in range(4)]
    NP = nPlo + nPhi
    NA = sum(nAlo) + sum(nAhi)
    dims = {"nPlo": nPlo, "nPhi": nPhi, "nAlo": tuple(nAlo), "nAhi": tuple(nAhi),
            "vAlo": tuple(vAlo), "vAhi": tuple(vAhi),
            "vPlo": vPlo, "vPhi": vPhi}

    # planes tensor column layout (bf16 [128, PL_N])
    PL_PP = 0
    PL_PC = PL_PP + 8 * NP
    PL_PB = PL_PC + 8 * NA
    PL_LEMB = PL_PB + 8 * NA
    PL_WA2B = PL_LEMB + 32
    PL_SMROW = PL_WA2B + H
    PL_IDENT = PL_SMROW + S
    PL_N = PL_IDENT + 128
    dims["PL"] = (PL_PP, PL_PC, PL_PB, PL_LEMB, PL_WA2B, PL_SMROW, PL_IDENT, PL_N)

    in_maps = []
    for core in range(NCORES):
        b, ch = core // 2, core % 2

        # pd sections: idx-0 pad to full chunks (prefix sections, no -1).
        # arg sections: idx-0 pad to the static valid count, -1 to chunk end.
        def padsec(lst, vcnt, nch, width):
            out = list(lst)
            while len(out) < vcnt:
                out.append((0,) + (0,) * (width - 2) + (0.0,))
            while len(out) < nch * 128:
                out.append((-1,) + (0,) * (width - 2) + (0.0,))
            return out

        pl = padsec(core_pd[core][0], vPlo, nPlo, 3)
        ph = padsec(core_pd[core][1], vPhi, nPhi, 3)
        argsec = []
        for cb in range(4):
            argsec.append((padsec(core_arg[core][cb][0], vAlo[cb], nAlo[cb], 4),
                           padsec(core_arg[core][cb][1], vAhi[cb], nAhi[cb], 4)))

        # idx stream, instruction order: [pd_lo], [pd_hi], (a_lo_cb, a_hi_cb)...
        idx_flat = [t[0] for t in pl] + [t[0] for t in ph]
        for cb in range(4):
            idx_flat += [t[0] for t in argsec[cb][0]]
            idx_flat += [t[0] for t in argsec[cb][1]]
        idxw = _wrap_idx(np.asarray(idx_flat, np.int64))

        # planes
        planes = np.zeros((128, PL_N), np.float32)
        # pd planes: chunk k global over [pd_lo chunks, pd_hi chunks]
        for k, lst in ((0, pl), (nPlo, ph)):
            for i, (idv, n, w_) in enumerate(lst):
                if idv < 0:
                    continue
                planes[i % 128, PL_PP + 8 * (k + i // 128) + n] = w_
        # arg planes: global chunk j over (cb: lo chunks, hi chunks)
        j0 = 0
        for cb in range(4):
            for lst in argsec[cb]:
                for i, (idv, c8, n, w_) in enumerate(lst):
                    if idv < 0:
                        continue
                    j = j0 + i // 128
                    p = i % 128
                    planes[p, PL_PC + 8 * j + c8] = w_
                    planes[p, PL_PB + n * NA + j] = 1.0
                j0 += len(lst) // 128
        planes[:, PL_LEMB:PL_LEMB + 32] = label_emb[ch * CH:(ch + 1) * CH, :].T
        planes[0:8, PL_WA2B:PL_WA2B + H] = np.broadcast_to(Wa2.reshape(1, H), (8, H))
        spl = max(1, int(pred_end[b] - pred_start[b]))
        pos = np.arange(S)
        smr = ((pos >= pred_start[b]) & (pos < pred_end[b])).astype(np.float32) / spl
        planes[:, PL_SMROW:PL_SMROW + S] = smr[None, :]
        planes[:, PL_IDENT:PL_IDENT + 128] = np.eye(128, dtype=np.float32)

        cf = np.zeros((128, CF_N), np.float32)
        cf[:, CF_ONES:CF_ONES + 8] = 1.0
        cf[0:8, CF_ID8:CF_ID8 + 8] = np.eye(8, dtype=np.float32)
        cf[0:8, CF_SCOL] = core_scol[core]
        cf[0:8, CF_BA2] = float(ba2[0])
        cf[:, CF_B2B] = float(b2[0])
        cf[0, CF_B1R:CF_B1R + DH] = b1
        cf[0:8, CF_O8x128:CF_O8x128 + 128] = 1.0

        xT = _pack(np.ascontiguousarray(x[b].T), H, S).astype(BF)  # [128, 6*256]

        in_maps.append({
            "wlo": wlo,
            "whi": whi,
            "idx": idxw,
            "planes": planes.astype(BF),
            "cf32": cf,
            "xT": xT,
            "wa1": wa1_p,
            "w1x": w1x_p,
            "w1l": w1l_p,
            "w1p": w1p_p,
            "w2": w2_p,
        })
    return dims, in_maps


def build_program(dims):
    nPlo, nPhi = dims["nPlo"], dims["nPhi"]
    nAlo, nAhi = dims["nAlo"], dims["nAhi"]
    NP = nPlo + nPhi
    NA = sum(nAlo) + sum(nAhi)
    (PL_PP, PL_PC, PL_PB, PL_LEMB, PL_WA2B, PL_SMROW, PL_IDENT, PL_N) = dims["PL"]

    nc = bacc.Bacc("TRN2", target_bir_lowering=False, debug=False,
                   num_devices=NCORES, dynamic_dma_scratch_size=65536,
                   num_swdge_queues=4)

    dt = nc.dram_tensor
    t_wlo = dt("wlo", [VSPLIT, ES], BF16, kind="ExternalInput")
    t_whi = dt("whi", [V - VSPLIT, ES], BF16, kind="ExternalInput")
    TCOL = (NP + NA) * 8
    t_idx = dt("idx", [128, TCOL], I16, kind="ExternalInput")
    t_planes = dt("planes", [128, PL_N], BF16, kind="ExternalInput")
    t_cf = dt("cf32", [128, CF_N], F32, kind="ExternalInput")
    t_xT = dt("xT", [128, HCH * S], BF16, kind="ExternalInput")
    t_wa1 = dt("wa1", [128, 9 * H], BF16, kind="ExternalInput")
    t_w1x = dt("w1x", [128, HCH * DH], BF16, kind="ExternalInput")
    t_w1l = dt("w1l", [128, 4 * DH], BF16, kind="ExternalInput")
    t_w1p = dt("w1p", [128, HCH * DH], BF16, kind="ExternalInput")
    t_w2 = dt("w2", [128, 3 * 32], BF16, kind="ExternalInput")
    t_out = dt("out", [16, 512], F32, kind="ExternalOutput")

    with tile.TileContext(nc) as tc:
        with tc.tile_pool(name="sb", bufs=1) as sb, \
             tc.tile_pool(name="sbt", bufs=6) as sbt, \
             tc.tile_pool(name="ppw", bufs=3, space="PSUM") as ppw, \
             tc.tile_pool(name="ppa", bufs=3, space="PSUM") as ppa, \
             tc.tile_pool(name="ppo", bufs=1, space="PSUM") as ppo:

            # ---------------- idx DMA + gathers first ----
            idx = sb.tile([128, TCOL], I16, tag="idx")
            nc.scalar.dma_start(out=idx[:], in_=t_idx[:])

            vAlo, vAhi = dims["vAlo"], dims["vAhi"]
            vPlo, vPhi = dims["vPlo"], dims["vPhi"]
            # hoist num_idxs_reg constants into registers (one MOVE per value)
            vreg = {}
            for v in set([vPlo, vPhi] + list(vAlo) + list(vAhi)):
                vreg[v] = nc.gpsimd.to_reg(v)

            def gather(tag, table, col0, nch, vcnt, q):
                g = sb.tile([128, nch * ES], BF16, tag=tag)
                nc.gpsimd.dma_gather(
                    out_ap=g[:, :].rearrange("p (c e) -> p c e", c=nch),
                    in_ap=table[:, :],
                    idxs_ap=idx[:, col0:col0 + nch * 8],
                    num_idxs=nch * 128,
                    num_idxs_reg=vreg[vcnt],
                    elem_size=ES,
                    queue_num=q,
                )
                return g

            # queue plan (emission parallel across contexts; q0 inline):
            #   q1: pd_lo, a0_lo, a2_hi ; q2: pd_hi, a0_hi wait->see map below
            # creation order i -> sem lane i%8; lanes lock to one queue, so
            # gathers 8,9 (a3lo/a3hi) must reuse the queues of 0,1 (pdlo/pdhi).
            # balanced to ~10 chunks/queue so no straggler queue extends the wave
            QMAP = {"pdlo": 1, "pdhi": 2,
                    "a0lo": 3, "a0hi": 0, "a1lo": 2, "a1hi": 3,
                    "a2lo": 0, "a2hi": 1, "a3lo": 1, "a3hi": 2}
            col = 0
            gpd_lo = gather("gpdl", t_wlo, col, nPlo, vPlo, QMAP["pdlo"])
            col += nPlo * 8
            gpd_hi = gather("gpdh", t_whi, col, nPhi, vPhi, QMAP["pdhi"])
            col += nPhi * 8
            garg = []
            for cb in range(4):
                glo = gather(f"gal{cb}", t_wlo, col, nAlo[cb], vAlo[cb],
                             QMAP[f"a{cb}lo"])
                col += nAlo[cb] * 8
                ghi = gather(f"gah{cb}", t_whi, col, nAhi[cb], vAhi[cb],
                             QMAP[f"a{cb}hi"])
                col += nAhi[cb] * 8
                garg.append((glo, ghi))
            goff = [(0, 0)] * 4

            # ---------------- remaining input DMAs ----------------
            # sync queue: early-need tensors
            xTall = sb.tile([128, HCH * S], BF16, tag="xT")
            nc.sync.dma_start(out=xTall[:], in_=t_xT[:])
            xT = [xTall[:, S * hc:S * (hc + 1)] for hc in range(HCH)]
            planes = sb.tile([128, PL_N], BF16, tag="planes")
            nc.sync.dma_start(out=planes[:], in_=t_planes[:])
            cf = sb.tile([128, CF_N], F32, tag="cf")
            nc.sync.dma_start(out=cf[:], in_=t_cf[:])
            wa1_all = sb.tile([128, 9 * H], BF16, tag="wa1")
            nc.sync.dma_start(out=wa1_all[:], in_=t_wa1[:])
            wa1 = [wa1_all[0:KA[i], H * i:H * (i + 1)] for i in range(9)]

            # scalar queue: rest, in need order
            w1x_all = sb.tile([128, HCH * DH], BF16, tag="w1x")
            nc.scalar.dma_start(out=w1x_all[:], in_=t_w1x[:])
            w1x = [w1x_all[:, DH * i:DH * (i + 1)] for i in range(HCH)]
            w1p_all = sb.tile([128, HCH * DH], BF16, tag="w1p")
            nc.scalar.dma_start(out=w1p_all[:], in_=t_w1p[:])
            w1p = [w1p_all[:, DH * i:DH * (i + 1)] for i in range(HCH)]
            w2_all = sb.tile([128, 3 * 32], BF16, tag="w2")
            nc.scalar.dma_start(out=w2_all[:], in_=t_w2[:])
            w2c = [w2_all[0:(d1 - d0), 32 * i:32 * (i + 1)] for i, (d0, d1) in enumerate(DCH)]
            w1l_all = sb.tile([128, 4 * DH], BF16, tag="w1l")
            nc.scalar.dma_start(out=w1l_all[:], in_=t_w1l[:])
            w1l = [w1l_all[0:KLR[i], DH * i:DH * (i + 1)] for i in range(4)]

            ident = planes[:, PL_IDENT:PL_IDENT + 128]
            smrow = planes[:, PL_SMROW:PL_SMROW + S]
            wa2b = planes[0:8, PL_WA2B:PL_WA2B + H]
            lembT = planes[:, PL_LEMB:PL_LEMB + 32]

            # ---------------- pd_agg + attention ----------------
            # predT via DVE masked reduce over s (xT only; runs first)
            attk = []
            for hc in range(HCH):
                prod = sbt.tile([128, S], BF16, tag="prod")
                nc.vector.tensor_tensor(out=prod[:], in0=xT[hc],
                                        in1=smrow, op=AL.mult)
                pT = sbt.tile([128, 1], F32, tag="pT")
                nc.vector.tensor_reduce(out=pT[:], in_=prod[:],
                                        axis=mybir.AxisListType.X, op=AL.add)
                a_ = sb.tile([128, 8], BF16, tag=f"attk{hc}")
                nc.vector.tensor_copy(out=a_[:], in_=pT[:, 0:1].to_broadcast([128, 8]))
                attk.append(a_)

            # hxT (xT + w1x only; PE stream ahead of gather-dependent work)
            hxTs = []
            for dc, (d0, d1) in enumerate(DCH):
                ds_ = d1 - d0
                hp_ = ppw.tile([ds_, S], F32, tag="w", name=f"hx{dc}")
                for hc in range(HCH):
                    nc.tensor.matmul(out=hp_[:], lhsT=w1x[hc][:, d0:d1], rhs=xT[hc],
                                     start=(hc == 0), stop=(hc == HCH - 1))
                hs = sb.tile([ds_, S], BF16, tag=f"hxT{dc}")
                (nc.scalar.copy if dc == 0 else nc.vector.tensor_copy)(
                    out=hs[:], in_=hp_[:])
                hxTs.append(hs)

            # hp row matmuls (predT + w1p)
            hprow = ppw.tile([1, DH], F32, tag="w", name="hprow")
            for i in range(HCH):
                nc.tensor.matmul(out=hprow[:], lhsT=attk[i][:, 0:1], rhs=w1p[i][:],
                                 start=(i == 0), stop=(i == HCH - 1), tile_position=(0, 0))

            # pd_agg accumulation
            pdps = ppa.tile([8, E], F32, tag="acc", name="pdps")
            k = 0
            for g, nch, vc in ((gpd_lo, nPlo, vPlo), (gpd_hi, nPhi, vPhi)):
                for c in range(nch):
                    vt = vc - 128 * (nch - 1) if c == nch - 1 else 128
                    nc.tensor.matmul(out=pdps[:],
                                     lhsT=planes[0:vt, PL_PP + 8 * (k + c):PL_PP + 8 * (k + c + 1)],
                                     rhs=g[0:vt, ES * c:ES * c + E],
                                     start=(k + c == 0), stop=(k + c == NP - 1))
                k += nch
            pd_agg = sb.tile([8, E], BF16, tag="pd_agg")
            nc.vector.tensor_copy(out=pd_agg[:], in_=pdps[:])
            for e in range(2):
                tp = ppw.tile([128, 8], BF16, tag="w", name=f"tpa{e}")
                nc.tensor.transpose(out=tp[:], in_=pd_agg[:, 128 * e:128 * (e + 1)],
                                    identity=ident[0:8, 0:8])
                a_ = sb.tile([128, 8], BF16, tag=f"attk{6 + e}")
                nc.vector.tensor_copy(out=a_[:], in_=tp[:])
                attk.append(a_)
            tp = ppw.tile([44, 8], BF16, tag="w", name="tpb")
            nc.tensor.transpose(out=tp[:], in_=pd_agg[:, 256:300], identity=ident[0:8, 0:8])
            a_ = sb.tile([45, 8], BF16, tag="attk8")
            nc.vector.memset(a_[:, :], 1.0)
            nc.vector.tensor_copy(out=a_[0:44, :], in_=tp[:])
            attk.append(a_)

            hidp = [ppw.tile([8, 384], F32, tag="w", name=f"hid{nb}") for nb in range(2)]
            for nb in range(2):
                for kk in range(9):
                    nc.tensor.matmul(out=hidp[nb][:], lhsT=attk[kk][:],
                                     rhs=wa1[kk][:, 384 * nb:384 * (nb + 1)],
                                     start=(kk == 0), stop=(kk == 8))
            hid = sb.tile([8, H], BF16, tag="hid")
            for nb in range(2):
                nc.scalar.activation(out=hid[:, 384 * nb:384 * (nb + 1)],
                                     in_=hidp[nb][:], func=AF.Relu)
            scr = sb.tile([8, H], BF16, tag="scr")
            nc.vector.tensor_tensor(out=scr[:], in0=hid[:], in1=wa2b[:], op=AL.mult)
            wraw = sb.tile([8, 1], F32, tag="wraw")
            nc.vector.tensor_reduce(out=wraw[:], in_=scr[:], axis=mybir.AxisListType.X,
                                    op=AL.add)
            wsb = sb.tile([8, 1], F32, tag="wsb")
            nc.vector.tensor_scalar(out=wsb[:], in0=wraw[:],
                                    scalar1=cf[0:8, CF_SCOL:CF_SCOL + 1],
                                    scalar2=cf[0:8, CF_BA2:CF_BA2 + 1],
                                    op0=AL.add, op1=AL.add)
            expc = sb.tile([8, 1], F32, tag="expc")
            nc.scalar.activation(out=expc[:], in_=wsb[:], func=AF.Exp)
            sps = ppw.tile([1, 1], F32, tag="w", name="sps")
            nc.tensor.matmul(out=sps[:], lhsT=expc[:], rhs=cf[0:8, CF_ONES:CF_ONES + 1],
                             start=True, stop=True)
            rs = sb.tile([1, 1], F32, tag="rs")
            nc.vector.reciprocal(out=rs[:], in_=sps[:])
            rbps = ppw.tile([8, 1], F32, tag="w", name="rbps")
            nc.tensor.matmul(out=rbps[:], lhsT=cf[0:1, CF_ONES:CF_ONES + 8], rhs=rs[:],
                             start=True, stop=True)
            wcol = sb.tile([8, 1], F32, tag="wcol")
            nc.vector.tensor_tensor(out=wcol[:], in0=expc[:], in1=rbps[:], op=AL.mult)

            # W8b[p, n] = w_n for all p
            wdiag = sb.tile([8, 8], F32, tag="wdiag")
            nc.vector.tensor_scalar(out=wdiag[:], in0=cf[0:8, CF_ID8:CF_ID8 + 8],
                                    scalar1=wcol[:], scalar2=None, op0=AL.mult)
            w8ps = ppw.tile([128, 8], F32, tag="w", name="w8ps")
            nc.tensor.matmul(out=w8ps[:], lhsT=cf[0:8, CF_O8x128:CF_O8x128 + 128],
                             rhs=wdiag[:], start=True, stop=True)
            w8b = sb.tile([128, 8], F32, tag="w8b")  # f32: tensor_scalar scalar
            nc.vector.tensor_copy(out=w8b[:], in_=w8ps[:])

            # wslotAll[p, j] = w_{sense(p,j)}
            wsa = sb.tile([128, NA], BF16, tag="wsa")
            nc.vector.tensor_scalar(out=wsa[:], in0=planes[:, PL_PB:PL_PB + NA],
                                    scalar1=w8b[:, 0:1], scalar2=None, op0=AL.mult)
            for n in range(1, 8):
                nc.vector.scalar_tensor_tensor(
                    out=wsa[:], in0=planes[:, PL_PB + n * NA:PL_PB + (n + 1) * NA],
                    scalar=w8b[:, n:n + 1], in1=wsa[:], op0=AL.mult, op1=AL.add)

            # all arg-agg lhsT planes in one DVE op:
            # lj_all[p, 8j+c] = planesC[p, 8j+c] * wsa[p, j]
            lj_all = sb.tile([128, 8 * NA], BF16, tag="lj_all")
            nc.vector.tensor_tensor(
                out=lj_all[:, :].rearrange("p (j c) -> p j c", j=NA),
                in0=planes[:, PL_PC:PL_PC + 8 * NA].rearrange("p (j c) -> p j c", j=NA),
                in1=wsa[:, :].unsqueeze(2).to_broadcast([128, NA, 8]),
                op=AL.mult)

            # ---------------- hpb -> hpbT ----------------
            hpb = sb.tile([1, DH], F32, tag="hpb")
            nc.vector.tensor_tensor(out=hpb[:], in0=hprow[:],
                                    in1=cf[0:1, CF_B1R:CF_B1R + DH], op=AL.add)
            hpbT = []
            for dc, (d0, d1) in enumerate(DCH):
                tp2 = ppw.tile([d1 - d0, 1], F32, tag="w", name=f"tp2{dc}")
                nc.tensor.transpose(out=tp2[:], in_=hpb[0:1, d0:d1],
                                    identity=cf[0:1, CF_ONES:CF_ONES + 1])
                hb = sb.tile([d1 - d0, 1], F32, tag=f"hpbT{dc}")
                nc.vector.tensor_copy(out=hb[:], in_=tp2[:])
                hpbT.append(hb)

            # ---------------- per class-block ----------------
            # global arg chunk index j, in (cb: lo, hi) order
            jbase = [0]
            for cb in range(4):
                jbase.append(jbase[-1] + nAlo[cb] + nAhi[cb])

            def emit_agg(cb):
                aw = ppa.tile([8, E], F32, tag="acc", name=f"aw{cb}")
                ncch = nAlo[cb] + nAhi[cb]
                for c in range(ncch):
                    j = jbase[cb] + c
                    if c < nAlo[cb]:
                        g, cc = garg[cb][0], goff[cb][0] + c
                        vtail = vAlo[cb] - 128 * (nAlo[cb] - 1) \
                            if c == nAlo[cb] - 1 else 128
                    else:
                        g, cc = garg[cb][1], goff[cb][1] + (c - nAlo[cb])
                        vtail = vAhi[cb] - 128 * (nAhi[cb] - 1) \
                            if c == ncch - 1 else 128
                    nc.tensor.matmul(out=aw[:],
                                     lhsT=lj_all[0:vtail, 8 * j:8 * (j + 1)],
                                     rhs=g[0:vtail, ES * cc:ES * cc + E],
                                     start=(c == 0), stop=(c == ncch - 1))
                return aw

            prev_group_end = [None, None]
            outp2 = None
            awq = [emit_agg(0)]
            for cb in range(4):
                cyc = cb // 2
                if cb % 2 == 0:
                    outp2 = [ppo.tile([128, 512], F32, tag=f"out{h}", name=f"outp{h}_{cyc}")
                             for h in range(2)]
                    prev_group_end = [None, None]

                aw = awq.pop(0)
                aws = sbt.tile([8, E], BF16, tag="aws")
                nc.vector.tensor_copy(out=aws[:], in_=aw[:])

                awsT = []
                for e, (e0, e1) in enumerate(DCH):
                    tp3 = ppw.tile([e1 - e0, 8], BF16, tag="w", name=f"tp3{cb}{e}")
                    nc.tensor.transpose(out=tp3[:], in_=aws[:, e0:e1], identity=ident[0:8, 0:8])
                    li = sbt.tile([e1 - e0, 8], BF16, tag=f"liTa{e}")
                    nc.vector.tensor_copy(out=li[:], in_=tp3[:])
                    awsT.append(li)

                hl = ppa.tile([8, DH], F32, tag="acc", name=f"hl{cb}")
                for kc in range(4):
                    lh = lembT[:, 8 * cb:8 * (cb + 1)] if kc == 0 else awsT[kc - 1][:]
                    nc.tensor.matmul(out=hl[:], lhsT=lh, rhs=w1l[kc][:],
                                     start=(kc == 0), stop=(kc == 3))
                hls = sbt.tile([8, DH], BF16, tag="hls")
                nc.vector.tensor_copy(out=hls[:], in_=hl[:])

                biasT = []
                for dc, (d0, d1) in enumerate(DCH):
                    tp4 = ppw.tile([d1 - d0, 8], BF16, tag="w", name=f"tp4{cb}{dc}")
                    nc.tensor.transpose(out=tp4[:], in_=hls[:, d0:d1], identity=ident[0:8, 0:8])
                    bt = sbt.tile([d1 - d0, 8], F32, tag=f"biasT{dc}")
                    nc.vector.tensor_scalar(out=bt[:], in0=tp4[:], scalar1=hpbT[dc][:],
                                            scalar2=None, op0=AL.add)
                    biasT.append(bt)

                # software pipeline: next block's aggregation fills PE bubbles
                # while ACT/DVE produce this block's relu tiles
                if cb == 0:
                    awq.append(emit_agg(1))

                # same-psum-bank pairs consecutively (h pattern 0,0,1,1):
                # per-MM bank cycling is the documented HAM re-throttle trigger
                for cl in (0, 2, 1, 3):
                    cp = 4 * cb + cl
                    h = cp % 2
                    row = 32 * ((cp % 8) // 2)
                    for dc, (d0, d1) in enumerate(DCH):
                        ds_ = d1 - d0
                        tt = sbt.tile([ds_, 512], BF16, tag="t", name=f"tt{cp}{dc}")
                        if dc != 1:
                            nc.scalar.activation(
                                out=tt[:, 0:256], in_=hxTs[dc][:], func=AF.Relu,
                                bias=biasT[dc][:, 2 * cl:2 * cl + 1])
                            nc.vector.tensor_scalar(
                                out=tt[:, 256:512], in0=hxTs[dc][:],
                                scalar1=biasT[dc][:, 2 * cl + 1:2 * cl + 2],
                                scalar2=0.0, op0=AL.add, op1=AL.max)
                        else:
                            nc.vector.tensor_scalar(
                                out=tt[:, 0:256], in0=hxTs[dc][:],
                                scalar1=biasT[dc][:, 2 * cl:2 * cl + 1],
                                scalar2=0.0, op0=AL.add, op1=AL.max)
                            nc.scalar.activation(
                                out=tt[:, 256:512], in_=hxTs[dc][:], func=AF.Relu,
                                bias=biasT[dc][:, 2 * cl + 1:2 * cl + 2])
                        mm = nc.tensor.matmul(out=outp2[h][row:row + 32, :], lhsT=w2c[dc][:],
                                              rhs=tt[:], start=(dc == 0), stop=(dc == 2),
                                              tile_position=(0, row), skip_group_check=True)
                        if dc == 0 and prev_group_end[h] is not None:
                            add_dep_helper(mm.ins, prev_group_end[h], sync=False,
                                           reason="serialize psum accumulation groups per bank")
                        if dc == 2:
                            prev_group_end[h] = mm.ins

                if cb + 2 < 4:
                    awq.append(emit_agg(cb + 2))

                if cb % 2 == 1:
                    for h in range(2):
                        osb = sb.tile([128, 512], F32, tag=f"osb{h}", name=f"osb{cyc}{h}")
                        nc.vector.tensor_scalar(out=osb[:], in0=outp2[h][:],
                                                scalar1=cf[:, CF_B2B:CF_B2B + 1],
                                                scalar2=None, op0=AL.add)
                        nc.sync.dma_start(out=t_out[8 * cyc + h:8 * cyc + 8:2, :],
                                          in_=osb[0:128:32, :])

    nc.compile()
    return nc


def assemble(results):
    logits = np.empty((B, S, C), np.float32)
    for core in range(NCORES):
        b, ch = core // 2, core % 2
        r = results[core]["out"].reshape(CH, S)
        logits[b, :, ch * CH:(ch + 1) * CH] = r.T
    return logits


_NC_CACHE = {}
LAST_RESULTS = None


def kernel(**inputs):
    global LAST_RESULTS
    dims, in_maps = prepare(inputs)
    key = (dims["nPlo"], dims["nPhi"], dims["nAlo"], dims["nAhi"])
    if key not in _NC_CACHE:
        _NC_CACHE[key] = build_program(dims)
    nc = _NC_CACHE[key]
    trace = bool(os.environ.get("KBENCH_TRACE"))
    res = run_bass_kernel_spmd(nc, in_maps, core_ids=list(range(NCORES)), trace=trace)
    LAST_RESULTS = res
    return assemble(res.results)

